# revision 37
# baseline (speedup 1.0000x reference)
"""LoRALinear kernel for Trainium2 (8 NeuronCores, SPMD data-parallel).

Computes out = x @ W.T + b + SCALE*((x@gA.T)@gB.T + (x@lA.T)@lB.T)
  x: [8, 2048, 1024] f32, W: [4096, 1024], b: [4096]
  gA/lA: [8, 1024], gB/lB: [4096, 8]  ->  out: [8, 2048, 4096] f32

Strategy: one batch of x per core. Host marshals pure layout/dtype only
(no module FLOPs): x -> x.T fp16 per core, W -> W.T fp16, b broadcast to
[128, 4096] f32, LoRA adapters stacked/pre-scaled as in the reference
low-rank-first formulation (A_cat = SCALE*[gA;lA] fp16, B_catT =
[gB.T;lB.T] fp16).

Device does all the math in ONE fused pipeline over o-tiles of 512:
  1. merge: wet[k, o] = W.T chunk + A_cat.T @ B_catT chunk (rank-16
     matmul into PSUM, DVE add evicts onto the DMA'd W.T chunk in a
     3-deep ring). The merge of o-tile ot+1 is sprinkled between the
     s-tile groups of main(ot) so neither the PE nor the Vector FIFO
     ever stalls at an o-tile boundary.
  2. main: for each of 16 s-tiles, 8 fp16 matmuls accumulate
     psum[s, o] over k; DVE adds bias f32 during eviction; out DMAs
     ride the Activation DGE queue so input prefetch (Sync queue)
     never queues behind stores.
A short burst of dependency-free warmup matmuls flips the PE HAM clock
gate to full rate while the first DMAs stream in. The PE stream is
~1088 back-to-back N=512 fp16 matmuls (~235us at 2.4GHz) with no phase
boundaries.

fp16 operand rounding gives ~3e-4 absmax relative error vs the f32
reference; accumulation stays f32 in PSUM.
"""
import numpy as np
from contextlib import ExitStack

import concourse.bass as bass
import concourse.tile as tile
from concourse import bacc, mybir
from concourse.bass import ts, ds
from concourse.bass_utils import run_bass_kernel_spmd

F32 = mybir.dt.float32
F16 = mybir.dt.float16

N_CORES = 8
B, S, DIN, DOUT, R = 8, 2048, 1024, 4096, 8
SCALE = 16.0 / 8
R2 = 2 * R

P = 128            # partition tile
OTILE = 512        # matmul moving free dim (one PSUM bank of f32)
KT = DIN // P      # 8 k-tiles
OT = DOUT // OTILE # 8 o-tiles
ST = S // P        # 16 s-tiles
SC = S // OTILE    # 4 s-chunks of 512 for x DMA granularity
WARMUP_MM = 4      # HAM warmup matmuls: bridge the PE from the engine
                   # preamble (~7.5us) to the first real matmul (~8.5us)
                   # so the stream starts without an idle window


def build_nc():
    nc = bacc.Bacc("TRN2", target_bir_lowering=False, debug=False,
                   num_devices=N_CORES)
    xT = nc.dram_tensor("xT", [DIN, S], F16, kind="ExternalInput").ap()
    WT = nc.dram_tensor("WT", [DIN, DOUT], F16, kind="ExternalInput").ap()
    bias = nc.dram_tensor("bias", [P, DOUT], F32, kind="ExternalInput").ap()
    A_cat = nc.dram_tensor("A_cat", [64, DIN], F16, kind="ExternalInput").ap()
    B_catT = nc.dram_tensor("B_catT", [64, DOUT], F16, kind="ExternalInput").ap()
    out = nc.dram_tensor("out", [S, DOUT], F32, kind="ExternalOutput").ap()

    with tile.TileContext(nc) as tc:
        with ExitStack() as ctx:
            const = ctx.enter_context(tc.tile_pool(name="const", bufs=1))
            xt_pool = ctx.enter_context(tc.tile_pool(name="xt", bufs=1))
            wet_pool = ctx.enter_context(tc.tile_pool(name="wet", bufs=3))
            out_pool = ctx.enter_context(tc.tile_pool(name="outp", bufs=4))
            pl_pool = ctx.enter_context(tc.tile_pool(name="pl", bufs=2, space="PSUM"))
            po_pool = ctx.enter_context(tc.tile_pool(name="po", bufs=6, space="PSUM"))

            # ---- HAM warmup: dependency-free matmuls run while the first
            # DMAs stream, so real matmuls start at 2.4GHz, not 1.2 ----
            junk = const.tile([P, OTILE], F16)
            nc.vector.memset(junk[:], 1.0)
            for i in range(WARMUP_MM):
                pw = po_pool.tile([P, OTILE], F32, tag="po")
                nc.tensor.matmul(pw[:], junk[:, :P], junk[:],
                                 start=True, stop=True)

            # ---- constants (input DMAs ride the Sync DGE queue in program
            # order; outputs use the Activation DGE queue so input prefetch
            # never queues behind result stores). The LoRA operands come
            # replicated at partition offsets 0 and 32 so pairs of rank-16
            # matmuls can run concurrently in separate 32-row PE strips. ----
            acat = const.tile([2 * 32, DIN], F16)
            nc.sync.dma_start(acat[:], A_cat)
            bcatt = const.tile([2 * 32, DOUT], F16)
            nc.sync.dma_start(bcatt[:, ts(0, OTILE)], B_catT[:, ts(0, OTILE)])
            bias_sb = const.tile([P, DOUT], F32)

            # x.T tiles: [128, k-tile, s-chunk 512] fp16, one DMA per s-chunk
            xts = [xt_pool.tile([P, KT, OTILE], F16, name=f"xt{sc}")
                   for sc in range(SC)]

            def issue_x_dma(sc, split=False):
                # split=True stages the k-tile 0/1 slice first so the
                # startup prologue's first main matmuls unblock early
                src = xT[:, ts(sc, OTILE)].rearrange("(kt p) s -> p kt s", p=P)
                if split:
                    nc.sync.dma_start(xts[sc][:, 0:2, :], src[:, 0:2, :])
                    nc.sync.dma_start(xts[sc][:, 2:, :], src[:, 2:, :])
                else:
                    nc.sync.dma_start(xts[sc][:], src)

            def issue_wet_dma(ot, split=False):
                # DMA W.T chunk for o-tile `ot` into a fresh ring buffer.
                # split=True issues per-k-tile-pair quarters so the startup
                # merge chain starts as soon as the first 256KB lands.
                w = wet_pool.tile([P, KT, OTILE], F16, tag="wet",
                                  name=f"wet{ot}")
                src = WT[:, ts(ot, OTILE)].rearrange("(kt p) o -> p kt o", p=P)
                if split:
                    for q in range(KT // 2):
                        nc.sync.dma_start(w[:, 2 * q:2 * q + 2, :],
                                          src[:, 2 * q:2 * q + 2, :])
                else:
                    nc.sync.dma_start(w[:], src)
                return w

            def merge_pair(wet, ot, q):
                # merge LoRA into two W.T chunks: wet[kt] += acat.T @ bcatt
                # for kt = 2q, 2q+1, as two concurrently-executing rank-16
                # matmuls in PE row strips 0-31 / 32-63 (tile_position row
                # tiling), then two DVE adds.
                kts = (2 * q, 2 * q + 1)
                pls = [pl_pool.tile([P, OTILE], F32, tag="pl", name=f"pl{j}")
                       for j in range(2)]
                for j in range(2):
                    nc.tensor.matmul(pls[j][:],
                                     acat[ds(32 * j, R2), ts(kts[j], P)],
                                     bcatt[ds(32 * j, R2), ts(ot, OTILE)],
                                     start=True, stop=True,
                                     tile_position=(32 * j, 0))
                for j in range(2):
                    nc.vector.tensor_tensor(wet[:, kts[j], :],
                                            wet[:, kts[j], :], pls[j][:],
                                            mybir.AluOpType.add)

            wet_cur = issue_wet_dma(0, split=True)
            issue_x_dma(0, split=True)
            nc.sync.dma_start(bias_sb[:, ts(0, OTILE)], bias[:, ts(0, OTILE)])
            wet_next = issue_wet_dma(1)
            nc.sync.dma_start(bcatt[:, ds(OTILE, DOUT - OTILE)],
                              B_catT[:, ds(OTILE, DOUT - OTILE)])
            for sc in range(1, SC):
                issue_x_dma(sc)

            def evict(po, st, ot):
                osb = out_pool.tile([P, OTILE], F32, tag="osb",
                                    name=f"osb{ot}_{st}")
                nc.vector.tensor_tensor(osb[:], po[:],
                                        bias_sb[:, ts(ot, OTILE)],
                                        mybir.AluOpType.add)
                nc.scalar.dma_start(out[ts(st, P), ts(ot, OTILE)], osb[:])

            # ---- startup prologue: kt-outer over the first 4 s-tile
            # groups with the merge(0) pairs interleaved. The PE stays busy
            # (2 LoRA + 8 main mms per pair-step) while the DVE add chain
            # and the remaining startup DMAs stream -> HAM never throttles.
            pos0 = [po_pool.tile([P, OTILE], F32, tag="po", name=f"po0_{st}")
                    for st in range(4)]
            for q in range(KT // 2):
                merge_pair(wet_cur, 0, q)
                for kt in (2 * q, 2 * q + 1):
                    for st in range(4):
                        nc.tensor.matmul(pos0[st][:],
                                         xts[0][:, kt, ts(st, P)],
                                         wet_cur[:, kt, :],
                                         start=(kt == 0), stop=(kt == KT - 1))
            for st in range(4):
                evict(pos0[st], st, 0)

            # ---- fused, software-pipelined merge + main loop over o-tiles.
            # The merge of o-tile ot+1 (4 LoRA matmul pairs + DVE adds) is
            # sprinkled between the s-tile groups of main(ot), so the pl
            # ring drains between pairs and the DVE adds interleave with
            # already-drained bias evictions. ----
            for ot in range(OT):
                if ot + 2 < OT:
                    wet_fut = issue_wet_dma(ot + 2)
                if ot + 1 < OT:
                    nc.sync.dma_start(bias_sb[:, ts(ot + 1, OTILE)],
                                      bias[:, ts(ot + 1, OTILE)])
                st0 = 4 if ot == 0 else 0
                merge_at = {st0 + 1 + 3 * q: q for q in range(4)}
                # main: out[s, o] = x @ wet + bias
                for st in range(st0, ST):
                    if ot + 1 < OT and st in merge_at:
                        merge_pair(wet_next, ot + 1, merge_at[st])
                    sc, sp = st // 4, st % 4
                    po = po_pool.tile([P, OTILE], F32, tag="po")
                    for kt in range(KT):
                        nc.tensor.matmul(po[:], xts[sc][:, kt, ts(sp, P)],
                                         wet_cur[:, kt, :],
                                         start=(kt == 0), stop=(kt == KT - 1))
                    evict(po, st, ot)
                if ot + 1 < OT:
                    wet_cur = wet_next
                if ot + 2 < OT:
                    wet_next = wet_fut

    nc.compile()
    return nc


_NC_CACHE = None


def _get_nc():
    global _NC_CACHE
    if _NC_CACHE is None:
        _NC_CACHE = build_nc()
    return _NC_CACHE


def make_in_maps(x, W, b, global_A, global_B, local_A, local_B):
    x16 = np.asarray(x, dtype=np.float32).astype(np.float16)
    xT = np.ascontiguousarray(x16.transpose(0, 2, 1))          # [B, DIN, S]
    WT = np.ascontiguousarray(
        np.asarray(W, dtype=np.float32).T).astype(np.float16)  # [DIN, DOUT]
    bias = np.ascontiguousarray(
        np.broadcast_to(np.asarray(b, dtype=np.float32), (P, DOUT)))
    a_cat = (SCALE * np.concatenate(
        [np.asarray(global_A), np.asarray(local_A)], axis=0)
    ).astype(np.float16)
    b_catT = np.concatenate(
        [np.asarray(global_B).T, np.asarray(local_B).T],
        axis=0).astype(np.float16)
    # replicate at partition offsets 0 and 32 for PE row-strip packing
    A_cat = np.zeros((64, DIN), dtype=np.float16)
    A_cat[0:R2] = a_cat
    A_cat[32:32 + R2] = a_cat
    B_catT = np.zeros((64, DOUT), dtype=np.float16)
    B_catT[0:R2] = b_catT
    B_catT[32:32 + R2] = b_catT
    return [
        {"xT": xT[i], "WT": WT, "bias": bias, "A_cat": A_cat,
         "B_catT": B_catT}
        for i in range(N_CORES)
    ]


def kernel(x, W, b, global_A, global_B, local_A, local_B):
    nc = _get_nc()
    in_maps = make_in_maps(x, W, b, global_A, global_B, local_A, local_B)
    res = run_bass_kernel_spmd(nc, in_maps, list(range(N_CORES))).results
    return np.stack([res[i]["out"] for i in range(N_CORES)], axis=0)


# revision 38
# speedup vs baseline: 1.0216x; 1.0216x over previous
"""LoRALinear kernel for Trainium2 (8 NeuronCores, SPMD data-parallel).

Computes out = x @ W.T + b + SCALE*((x@gA.T)@gB.T + (x@lA.T)@lB.T)
  x: [8, 2048, 1024] f32, W: [4096, 1024], b: [4096]
  gA/lA: [8, 1024], gB/lB: [4096, 8]  ->  out: [8, 2048, 4096] f32

Strategy: one batch of x per core. Host marshals pure layout/dtype only
(no module FLOPs): x -> x.T fp16 per core, W -> W.T fp16, b broadcast to
[128, 4096] f32, LoRA adapters stacked/pre-scaled as in the reference
low-rank-first formulation (A_cat = SCALE*[gA;lA] fp16, B_catT =
[gB.T;lB.T] fp16).

Device does all the math in ONE fused pipeline over o-tiles of 512:
  1. merge: wet[k, o] = W.T chunk + A_cat.T @ B_catT chunk (rank-16
     matmul into PSUM, DVE add evicts onto the DMA'd W.T chunk in a
     3-deep ring). The merge of o-tile ot+1 is sprinkled between the
     s-tile groups of main(ot) so neither the PE nor the Vector FIFO
     ever stalls at an o-tile boundary.
  2. main: for each of 16 s-tiles, 8 fp16 matmuls accumulate
     psum[s, o] over k; DVE adds bias f32 during eviction; out DMAs
     ride the Activation DGE queue so input prefetch (Sync queue)
     never queues behind stores.
A short burst of dependency-free warmup matmuls flips the PE HAM clock
gate to full rate while the first DMAs stream in. The PE stream is
~1088 back-to-back N=512 fp16 matmuls (~235us at 2.4GHz) with no phase
boundaries.

fp16 operand rounding gives ~3e-4 absmax relative error vs the f32
reference; accumulation stays f32 in PSUM.
"""
import numpy as np
from contextlib import ExitStack

import concourse.bass as bass
import concourse.tile as tile
from concourse import bacc, mybir
from concourse.bass import ts, ds
from concourse.bass_utils import run_bass_kernel_spmd

F32 = mybir.dt.float32
F16 = mybir.dt.float16

N_CORES = 8
B, S, DIN, DOUT, R = 8, 2048, 1024, 4096, 8
SCALE = 16.0 / 8
R2 = 2 * R

P = 128            # partition tile
OTILE = 512        # matmul moving free dim (one PSUM bank of f32)
KT = DIN // P      # 8 k-tiles
OT = DOUT // OTILE # 8 o-tiles
ST = S // P        # 16 s-tiles
SC = S // OTILE    # 4 s-chunks of 512 for x DMA granularity
WARMUP_MM = 4      # HAM warmup matmuls: bridge the PE from the engine
                   # preamble (~7.5us) to the first real matmul (~8.5us)
                   # so the stream starts without an idle window


def build_nc():
    nc = bacc.Bacc("TRN2", target_bir_lowering=False, debug=False,
                   num_devices=N_CORES)
    xT = nc.dram_tensor("xT", [DIN, S], F16, kind="ExternalInput").ap()
    WT = nc.dram_tensor("WT", [DIN, DOUT], F16, kind="ExternalInput").ap()
    bias = nc.dram_tensor("bias", [P, DOUT], F32, kind="ExternalInput").ap()
    A_cat = nc.dram_tensor("A_cat", [64, DIN], F16, kind="ExternalInput").ap()
    B_catT = nc.dram_tensor("B_catT", [64, DOUT], F16, kind="ExternalInput").ap()
    out = nc.dram_tensor("out", [S, DOUT], F32, kind="ExternalOutput").ap()

    with tile.TileContext(nc) as tc:
        with ExitStack() as ctx:
            const = ctx.enter_context(tc.tile_pool(name="const", bufs=1))
            xt_pool = ctx.enter_context(tc.tile_pool(name="xt", bufs=1))
            wet_pool = ctx.enter_context(tc.tile_pool(name="wet", bufs=3))
            out_pool = ctx.enter_context(tc.tile_pool(name="outp", bufs=4))
            pl_pool = ctx.enter_context(tc.tile_pool(name="pl", bufs=2, space="PSUM"))
            po_pool = ctx.enter_context(tc.tile_pool(name="po", bufs=6, space="PSUM"))

            # ---- HAM warmup: dependency-free matmuls run while the first
            # DMAs stream, so real matmuls start at 2.4GHz, not 1.2 ----
            junk = const.tile([P, OTILE], F16)
            nc.vector.memset(junk[:], 1.0)
            for i in range(WARMUP_MM):
                pw = po_pool.tile([P, OTILE], F32, tag="po")
                nc.tensor.matmul(pw[:], junk[:, :P], junk[:],
                                 start=True, stop=True)

            # ---- constants (input DMAs ride the Sync DGE queue in program
            # order; outputs use the Activation DGE queue so input prefetch
            # never queues behind result stores). The LoRA operands come
            # replicated at partition offsets 0 and 32 so pairs of rank-16
            # matmuls can run concurrently in separate 32-row PE strips. ----
            acat = const.tile([2 * 32, DIN], F16)
            nc.sync.dma_start(acat[:], A_cat)
            bcatt = const.tile([2 * 32, DOUT], F16)
            nc.sync.dma_start(bcatt[:, ts(0, OTILE)], B_catT[:, ts(0, OTILE)])
            bias_sb = const.tile([P, DOUT], F32)

            # x.T tiles: [128, k-tile, s-chunk 512] fp16, one DMA per s-chunk
            xts = [xt_pool.tile([P, KT, OTILE], F16, name=f"xt{sc}")
                   for sc in range(SC)]

            def issue_x_dma(sc, split=False):
                # split=True stages the k-tile 0/1 slice first so the
                # startup prologue's first main matmuls unblock early
                src = xT[:, ts(sc, OTILE)].rearrange("(kt p) s -> p kt s", p=P)
                if split:
                    nc.sync.dma_start(xts[sc][:, 0:2, :], src[:, 0:2, :])
                    nc.sync.dma_start(xts[sc][:, 2:, :], src[:, 2:, :])
                else:
                    nc.sync.dma_start(xts[sc][:], src)

            def issue_wet_dma(ot, split=False):
                # DMA W.T chunk for o-tile `ot` into a fresh ring buffer.
                # split=True issues per-k-tile-pair quarters so the startup
                # merge chain starts as soon as the first 256KB lands.
                w = wet_pool.tile([P, KT, OTILE], F16, tag="wet",
                                  name=f"wet{ot}")
                src = WT[:, ts(ot, OTILE)].rearrange("(kt p) o -> p kt o", p=P)
                if split:
                    for q in range(KT // 2):
                        nc.sync.dma_start(w[:, 2 * q:2 * q + 2, :],
                                          src[:, 2 * q:2 * q + 2, :])
                else:
                    nc.sync.dma_start(w[:], src)
                return w

            def merge_pair(wet, ot, q):
                # merge LoRA into two W.T chunks: wet[kt] += acat.T @ bcatt
                # for kt = 2q, 2q+1, as two concurrently-executing rank-16
                # matmuls in PE row strips 0-31 / 32-63 (tile_position row
                # tiling), then two DVE adds.
                kts = (2 * q, 2 * q + 1)
                pls = [pl_pool.tile([P, OTILE], F32, tag="pl", name=f"pl{j}")
                       for j in range(2)]
                for j in range(2):
                    nc.tensor.matmul(pls[j][:],
                                     acat[ds(32 * j, R2), ts(kts[j], P)],
                                     bcatt[ds(32 * j, R2), ts(ot, OTILE)],
                                     start=True, stop=True,
                                     tile_position=(32 * j, 0))
                for j in range(2):
                    nc.vector.tensor_tensor(wet[:, kts[j], :],
                                            wet[:, kts[j], :], pls[j][:],
                                            mybir.AluOpType.add)

            wet_cur = issue_wet_dma(0, split=True)
            issue_x_dma(0, split=True)
            nc.sync.dma_start(bias_sb[:, ts(0, OTILE)], bias[:, ts(0, OTILE)])
            nc.sync.dma_start(bcatt[:, ts(1, OTILE)], B_catT[:, ts(1, OTILE)])
            issue_x_dma(1)
            wet_next = issue_wet_dma(1)
            issue_x_dma(2)
            nc.sync.dma_start(bcatt[:, ds(2 * OTILE, DOUT - 2 * OTILE)],
                              B_catT[:, ds(2 * OTILE, DOUT - 2 * OTILE)])
            issue_x_dma(3)

            def evict(po, st, ot):
                osb = out_pool.tile([P, OTILE], F32, tag="osb",
                                    name=f"osb{ot}_{st}")
                nc.vector.tensor_tensor(osb[:], po[:],
                                        bias_sb[:, ts(ot, OTILE)],
                                        mybir.AluOpType.add)
                nc.scalar.dma_start(out[ts(st, P), ts(ot, OTILE)], osb[:])

            # ---- startup prologue: kt-outer over the first 4 s-tile
            # groups with the merge(0) pairs interleaved. The PE stays busy
            # (2 LoRA + 8 main mms per pair-step) while the DVE add chain
            # and the remaining startup DMAs stream -> HAM never throttles.
            pos0 = [po_pool.tile([P, OTILE], F32, tag="po", name=f"po0_{st}")
                    for st in range(4)]
            for q in range(KT // 2):
                merge_pair(wet_cur, 0, q)
                for kt in (2 * q, 2 * q + 1):
                    for st in range(4):
                        nc.tensor.matmul(pos0[st][:],
                                         xts[0][:, kt, ts(st, P)],
                                         wet_cur[:, kt, :],
                                         start=(kt == 0), stop=(kt == KT - 1))
            for st in range(4):
                evict(pos0[st], st, 0)

            # ---- fused, software-pipelined merge + main loop over o-tiles.
            # The merge of o-tile ot+1 (4 LoRA matmul pairs + DVE adds) is
            # sprinkled between the s-tile groups of main(ot), so the pl
            # ring drains between pairs and the DVE adds interleave with
            # already-drained bias evictions. ----
            for ot in range(OT):
                if ot + 2 < OT:
                    wet_fut = issue_wet_dma(ot + 2)
                if ot + 1 < OT:
                    nc.sync.dma_start(bias_sb[:, ts(ot + 1, OTILE)],
                                      bias[:, ts(ot + 1, OTILE)])
                st0 = 4 if ot == 0 else 0
                merge_at = {st0 + 1 + 3 * q: q for q in range(4)}
                # main: out[s, o] = x @ wet + bias
                for st in range(st0, ST):
                    if ot + 1 < OT and st in merge_at:
                        merge_pair(wet_next, ot + 1, merge_at[st])
                    sc, sp = st // 4, st % 4
                    po = po_pool.tile([P, OTILE], F32, tag="po")
                    for kt in range(KT):
                        nc.tensor.matmul(po[:], xts[sc][:, kt, ts(sp, P)],
                                         wet_cur[:, kt, :],
                                         start=(kt == 0), stop=(kt == KT - 1))
                    evict(po, st, ot)
                if ot + 1 < OT:
                    wet_cur = wet_next
                if ot + 2 < OT:
                    wet_next = wet_fut

    nc.compile()
    return nc


_NC_CACHE = None


def _get_nc():
    global _NC_CACHE
    if _NC_CACHE is None:
        _NC_CACHE = build_nc()
    return _NC_CACHE


def make_in_maps(x, W, b, global_A, global_B, local_A, local_B):
    x16 = np.asarray(x, dtype=np.float32).astype(np.float16)
    xT = np.ascontiguousarray(x16.transpose(0, 2, 1))          # [B, DIN, S]
    WT = np.ascontiguousarray(
        np.asarray(W, dtype=np.float32).T).astype(np.float16)  # [DIN, DOUT]
    bias = np.ascontiguousarray(
        np.broadcast_to(np.asarray(b, dtype=np.float32), (P, DOUT)))
    a_cat = (SCALE * np.concatenate(
        [np.asarray(global_A), np.asarray(local_A)], axis=0)
    ).astype(np.float16)
    b_catT = np.concatenate(
        [np.asarray(global_B).T, np.asarray(local_B).T],
        axis=0).astype(np.float16)
    # replicate at partition offsets 0 and 32 for PE row-strip packing
    A_cat = np.zeros((64, DIN), dtype=np.float16)
    A_cat[0:R2] = a_cat
    A_cat[32:32 + R2] = a_cat
    B_catT = np.zeros((64, DOUT), dtype=np.float16)
    B_catT[0:R2] = b_catT
    B_catT[32:32 + R2] = b_catT
    return [
        {"xT": xT[i], "WT": WT, "bias": bias, "A_cat": A_cat,
         "B_catT": B_catT}
        for i in range(N_CORES)
    ]


def kernel(x, W, b, global_A, global_B, local_A, local_B):
    nc = _get_nc()
    in_maps = make_in_maps(x, W, b, global_A, global_B, local_A, local_B)
    res = run_bass_kernel_spmd(nc, in_maps, list(range(N_CORES))).results
    return np.stack([res[i]["out"] for i in range(N_CORES)], axis=0)


# revision 40
# speedup vs baseline: 1.0247x; 1.0031x over previous
"""LoRALinear kernel for Trainium2 (8 NeuronCores, SPMD data-parallel).

Computes out = x @ W.T + b + SCALE*((x@gA.T)@gB.T + (x@lA.T)@lB.T)
  x: [8, 2048, 1024] f32, W: [4096, 1024], b: [4096]
  gA/lA: [8, 1024], gB/lB: [4096, 8]  ->  out: [8, 2048, 4096] f32

Strategy: one batch of x per core. Host marshals pure layout/dtype only
(no module FLOPs): x -> x.T fp16 per core, W -> W.T fp16, b broadcast to
[128, 4096] f32, LoRA adapters stacked/pre-scaled as in the reference
low-rank-first formulation (A_cat = SCALE*[gA;lA] fp16, B_catT =
[gB.T;lB.T] fp16).

Device does all the math in ONE fused pipeline over o-tiles of 512:
  1. merge: wet[k, o] = W.T chunk + A_cat.T @ B_catT chunk. The rank-16
     LoRA matmuls run as PAIRS in separate 32-row PE strips
     (tile_position row tiling, operands host-replicated at partition
     offsets 0/32) so each pair costs one matmul slot; DVE adds evict
     onto the DMA'd W.T chunk in a 3-deep ring. The merge of o-tile
     ot+1 is sprinkled between the s-tile groups of main(ot) so
     neither the PE nor the Vector FIFO stalls at o-tile boundaries.
  2. main: for each of 16 s-tiles, 8 fp16 matmuls accumulate
     psum[s, o] over k; DVE adds bias f32 during eviction; out DMAs
     ride the Activation DGE queue so input prefetch (Sync queue)
     never queues behind stores.
Startup: a short dependency-free warmup burst flips the PE HAM clock
gate to 2.4GHz while the first (finely-staged, need-ordered) DMAs
stream in, and a kt-outer prologue interleaves the merge(0) chain with
the first four s-tile groups so the PE never idles long enough to
re-throttle. Steady state measures at the N=512 issue floor
(~216ns/matmul); HW exec ~259us vs the ~221us pure-matmul bound.

fp16 operand rounding gives ~3e-4 absmax relative error vs the f32
reference; accumulation stays f32 in PSUM.
"""
import numpy as np
from contextlib import ExitStack

import concourse.bass as bass
import concourse.tile as tile
from concourse import bacc, mybir
from concourse.bass import ts, ds
from concourse.bass_utils import run_bass_kernel_spmd

F32 = mybir.dt.float32
F16 = mybir.dt.float16

N_CORES = 8
B, S, DIN, DOUT, R = 8, 2048, 1024, 4096, 8
SCALE = 16.0 / 8
R2 = 2 * R

P = 128            # partition tile
OTILE = 512        # matmul moving free dim (one PSUM bank of f32)
KT = DIN // P      # 8 k-tiles
OT = DOUT // OTILE # 8 o-tiles
ST = S // P        # 16 s-tiles
SC = S // OTILE    # 4 s-chunks of 512 for x DMA granularity
WARMUP_MM = 12     # HAM warmup matmuls: bridge the PE from the engine
                   # preamble (~7.5us) until the prologue's inputs have all
                   # landed (~12.5us); HAM flips to 2.4GHz at ~11us so the
                   # real stream runs warm from its first matmul


def build_nc():
    nc = bacc.Bacc("TRN2", target_bir_lowering=False, debug=False,
                   num_devices=N_CORES)
    xT = nc.dram_tensor("xT", [DIN, S], F16, kind="ExternalInput").ap()
    WT = nc.dram_tensor("WT", [DIN, DOUT], F16, kind="ExternalInput").ap()
    bias = nc.dram_tensor("bias", [P, DOUT], F32, kind="ExternalInput").ap()
    A_cat = nc.dram_tensor("A_cat", [64, DIN], F16, kind="ExternalInput").ap()
    B_catT = nc.dram_tensor("B_catT", [64, DOUT], F16, kind="ExternalInput").ap()
    out = nc.dram_tensor("out", [S, DOUT], F32, kind="ExternalOutput").ap()

    with tile.TileContext(nc) as tc:
        with ExitStack() as ctx:
            const = ctx.enter_context(tc.tile_pool(name="const", bufs=1))
            xt_pool = ctx.enter_context(tc.tile_pool(name="xt", bufs=1))
            wet_pool = ctx.enter_context(tc.tile_pool(name="wet", bufs=3))
            out_pool = ctx.enter_context(tc.tile_pool(name="outp", bufs=4))
            pl_pool = ctx.enter_context(tc.tile_pool(name="pl", bufs=2, space="PSUM"))
            po_pool = ctx.enter_context(tc.tile_pool(name="po", bufs=6, space="PSUM"))

            # ---- HAM warmup: dependency-free matmuls run while the first
            # DMAs stream, so real matmuls start at 2.4GHz, not 1.2 ----
            junk = const.tile([P, OTILE], F16)
            nc.vector.memset(junk[:], 1.0)
            for i in range(WARMUP_MM):
                pw = po_pool.tile([P, OTILE], F32, tag="po")
                nc.tensor.matmul(pw[:], junk[:, :P], junk[:],
                                 start=True, stop=True)

            # ---- constants (input DMAs ride the Sync DGE queue in program
            # order; outputs use the Activation DGE queue so input prefetch
            # never queues behind result stores). The LoRA operands come
            # replicated at partition offsets 0 and 32 so pairs of rank-16
            # matmuls can run concurrently in separate 32-row PE strips. ----
            acat = const.tile([2 * 32, DIN], F16)
            nc.sync.dma_start(acat[:], A_cat)
            bcatt = const.tile([2 * 32, DOUT], F16)
            nc.sync.dma_start(bcatt[:, ts(0, OTILE)], B_catT[:, ts(0, OTILE)])
            bias_sb = const.tile([P, DOUT], F32)

            # x.T tiles: [128, k-tile, s-chunk 512] fp16, one DMA per s-chunk
            xts = [xt_pool.tile([P, KT, OTILE], F16, name=f"xt{sc}")
                   for sc in range(SC)]

            def issue_x_dma(sc, split=False):
                # split=True stages the k-tile 0/1 slice first so the
                # startup prologue's first main matmuls unblock early
                src = xT[:, ts(sc, OTILE)].rearrange("(kt p) s -> p kt s", p=P)
                if split:
                    nc.sync.dma_start(xts[sc][:, 0:2, :], src[:, 0:2, :])
                    nc.sync.dma_start(xts[sc][:, 2:, :], src[:, 2:, :])
                else:
                    nc.sync.dma_start(xts[sc][:], src)

            def issue_wet_dma(ot, split=False):
                # DMA W.T chunk for o-tile `ot` into a fresh ring buffer.
                # split=True issues per-k-tile-pair quarters so the startup
                # merge chain starts as soon as the first 256KB lands.
                w = wet_pool.tile([P, KT, OTILE], F16, tag="wet",
                                  name=f"wet{ot}")
                src = WT[:, ts(ot, OTILE)].rearrange("(kt p) o -> p kt o", p=P)
                if split:
                    for q in range(KT // 2):
                        nc.sync.dma_start(w[:, 2 * q:2 * q + 2, :],
                                          src[:, 2 * q:2 * q + 2, :])
                else:
                    nc.sync.dma_start(w[:], src)
                return w

            def merge_pair(wet, ot, q):
                # merge LoRA into two W.T chunks: wet[kt] += acat.T @ bcatt
                # for kt = 2q, 2q+1, as two concurrently-executing rank-16
                # matmuls in PE row strips 0-31 / 32-63 (tile_position row
                # tiling), then two DVE adds.
                kts = (2 * q, 2 * q + 1)
                pls = [pl_pool.tile([P, OTILE], F32, tag="pl", name=f"pl{j}")
                       for j in range(2)]
                for j in range(2):
                    nc.tensor.matmul(pls[j][:],
                                     acat[ds(32 * j, R2), ts(kts[j], P)],
                                     bcatt[ds(32 * j, R2), ts(ot, OTILE)],
                                     start=True, stop=True,
                                     tile_position=(32 * j, 0))
                for j in range(2):
                    nc.vector.tensor_tensor(wet[:, kts[j], :],
                                            wet[:, kts[j], :], pls[j][:],
                                            mybir.AluOpType.add)

            wet_cur = issue_wet_dma(0, split=True)
            issue_x_dma(0, split=True)
            nc.sync.dma_start(bias_sb[:, ts(0, OTILE)], bias[:, ts(0, OTILE)])
            nc.sync.dma_start(bcatt[:, ts(1, OTILE)], B_catT[:, ts(1, OTILE)])
            issue_x_dma(1)
            wet_next = issue_wet_dma(1)
            issue_x_dma(2)
            nc.sync.dma_start(bcatt[:, ds(2 * OTILE, DOUT - 2 * OTILE)],
                              B_catT[:, ds(2 * OTILE, DOUT - 2 * OTILE)])
            issue_x_dma(3)

            def evict(po, st, ot):
                osb = out_pool.tile([P, OTILE], F32, tag="osb",
                                    name=f"osb{ot}_{st}")
                nc.vector.tensor_tensor(osb[:], po[:],
                                        bias_sb[:, ts(ot, OTILE)],
                                        mybir.AluOpType.add)
                nc.scalar.dma_start(out[ts(st, P), ts(ot, OTILE)], osb[:])

            # ---- startup prologue: kt-outer over the first 4 s-tile
            # groups with the merge(0) pairs interleaved. The PE stays busy
            # (2 LoRA + 8 main mms per pair-step) while the DVE add chain
            # and the remaining startup DMAs stream -> HAM never throttles.
            pos0 = [po_pool.tile([P, OTILE], F32, tag="po", name=f"po0_{st}")
                    for st in range(4)]
            for q in range(KT // 2):
                merge_pair(wet_cur, 0, q)
                for kt in (2 * q, 2 * q + 1):
                    for st in range(4):
                        nc.tensor.matmul(pos0[st][:],
                                         xts[0][:, kt, ts(st, P)],
                                         wet_cur[:, kt, :],
                                         start=(kt == 0), stop=(kt == KT - 1))
            for st in range(4):
                evict(pos0[st], st, 0)

            # ---- fused, software-pipelined merge + main loop over o-tiles.
            # The merge of o-tile ot+1 (4 LoRA matmul pairs + DVE adds) is
            # sprinkled between the s-tile groups of main(ot), so the pl
            # ring drains between pairs and the DVE adds interleave with
            # already-drained bias evictions. ----
            for ot in range(OT):
                if ot + 2 < OT:
                    wet_fut = issue_wet_dma(ot + 2)
                if ot + 1 < OT:
                    nc.sync.dma_start(bias_sb[:, ts(ot + 1, OTILE)],
                                      bias[:, ts(ot + 1, OTILE)])
                st0 = 4 if ot == 0 else 0
                merge_at = {st0 + 1 + 3 * q: q for q in range(4)}
                # main: out[s, o] = x @ wet + bias
                for st in range(st0, ST):
                    if ot + 1 < OT and st in merge_at:
                        merge_pair(wet_next, ot + 1, merge_at[st])
                    sc, sp = st // 4, st % 4
                    po = po_pool.tile([P, OTILE], F32, tag="po")
                    for kt in range(KT):
                        nc.tensor.matmul(po[:], xts[sc][:, kt, ts(sp, P)],
                                         wet_cur[:, kt, :],
                                         start=(kt == 0), stop=(kt == KT - 1))
                    evict(po, st, ot)
                if ot + 1 < OT:
                    wet_cur = wet_next
                if ot + 2 < OT:
                    wet_next = wet_fut

    nc.compile()
    return nc


_NC_CACHE = None


def _get_nc():
    global _NC_CACHE
    if _NC_CACHE is None:
        _NC_CACHE = build_nc()
    return _NC_CACHE


def make_in_maps(x, W, b, global_A, global_B, local_A, local_B):
    x16 = np.asarray(x, dtype=np.float32).astype(np.float16)
    xT = np.ascontiguousarray(x16.transpose(0, 2, 1))          # [B, DIN, S]
    WT = np.ascontiguousarray(
        np.asarray(W, dtype=np.float32).T).astype(np.float16)  # [DIN, DOUT]
    bias = np.ascontiguousarray(
        np.broadcast_to(np.asarray(b, dtype=np.float32), (P, DOUT)))
    a_cat = (SCALE * np.concatenate(
        [np.asarray(global_A), np.asarray(local_A)], axis=0)
    ).astype(np.float16)
    b_catT = np.concatenate(
        [np.asarray(global_B).T, np.asarray(local_B).T],
        axis=0).astype(np.float16)
    # replicate at partition offsets 0 and 32 for PE row-strip packing
    A_cat = np.zeros((64, DIN), dtype=np.float16)
    A_cat[0:R2] = a_cat
    A_cat[32:32 + R2] = a_cat
    B_catT = np.zeros((64, DOUT), dtype=np.float16)
    B_catT[0:R2] = b_catT
    B_catT[32:32 + R2] = b_catT
    return [
        {"xT": xT[i], "WT": WT, "bias": bias, "A_cat": A_cat,
         "B_catT": B_catT}
        for i in range(N_CORES)
    ]


def kernel(x, W, b, global_A, global_B, local_A, local_B):
    nc = _get_nc()
    in_maps = make_in_maps(x, W, b, global_A, global_B, local_A, local_B)
    res = run_bass_kernel_spmd(nc, in_maps, list(range(N_CORES))).results
    return np.stack([res[i]["out"] for i in range(N_CORES)], axis=0)
